# revision 35
# baseline (speedup 1.0000x reference)
"""Causal self-attention (B=4, T=2048, C=1024, H=16) on 8 trn2 NeuronCores.

Head-parallel tensor parallelism: each core owns 2 of 16 heads (its 384 rows
of Wqkv, its 128 columns of Wout); the 8 partial [C, B*T] outputs are summed
on the host (the all-reduce). HW-measured (NTFF) ~336us vs ~521us baseline.

Design:
 - bf16 datapath on SBUF (bf16 matmul = 1 cycle/row, same as fp32r, but DMA
   bytes and DVE element costs halve); PSUM/bias/softmax-denominator fp32.
 - attention_mask is all-ones per the problem spec (fill: ones) -> causal
   mask only, no pad machinery.
 - scores held transposed (S^T [tk, tq]) so the softmax denominator is a
   partition-dim sum that rides the PV matmul for free via an augmented
   stationary [v | ones]; normalize = 2 copies + reciprocal_approx_fast +
   2 multiplies on DVE (full reciprocal() costs 3.35us/call on HW; the
   approx custom op NaNs with partition-offset operands, hence the copies).
 - causal diagonal tiles trimmed: score matmuls, exp, and mask only cover
   the live q-range [128m, 512); PV reads the same range.
 - software pipelining: scores run ahead of PV; QKV + V-transpose units of
   batch b+1 and out-proj units of the previous q-block are interleaved
   between attention tiles so TensorE (roofline engine, ~275us busy) never
   starves; batched DMAs split across SP (x in), gpsimd/SWDGE (out).

Layouts (matmul contractions always on SBUF partitions):
  xT    [C, B*T]      x transposed on host, bf16
  qkv   [128, 3, T]   per batch; rows = 2 heads x 64 dims
  vAB   [128, i, 256] per key-tile: v_h0 | ones | ones | v_h1
  pyA   [Y_h0; r_h0]  PV psum (rows 64:128 = replicated rowsum)
  pyB   [r_h1; Y_h1]
  outT  [C, B*T]      partial output, bf16, summed across cores on host
"""

import numpy as np
from collections import deque
from contextlib import ExitStack

import concourse.bass as bass
import concourse.bacc as bacc
import concourse.mybir as mybir
import concourse.tile as tile
from concourse import bass_utils
from concourse.masks import make_identity

B, T, C = 4, 2048, 1024
H, D = 16, 64
NCORES = 8
HPC = H // NCORES            # heads per core = 2
CPC = HPC * D                # y-channels per core = 128
BT = B * T                   # 8192
F = 3 * CPC                  # qkv rows per core = 384
TQB = 512                    # tq block (matmul free dim)
NJ = T // TQB                # 4 tq blocks per batch
NKT = T // 128               # 16 tk tiles per batch
NCT = C // 128               # 8 contraction tiles for projections
FP32 = mybir.dt.float32
BF16 = mybir.dt.bfloat16
AF = mybir.ActivationFunctionType
ALU = mybir.AluOpType
SCALE = 1.0 / np.sqrt(D)
NPBF16 = mybir.dt.np(BF16)

_cached = {}

CFG = {
    "norm": "copies",       # "psum2" (recips straight off PSUM) | "copies"
    "mask_engine": "vector",  # "vector" | "split" (m<2 on pool)
    "interleave": True,
    "vt": "pe2",            # "pe2" (grouped PE transpose) | "dma" | "pe"
    "pair": True,           # process attention tiles two at a time
    "exp_merge": True,      # one strided exp call for trimmed diag tiles
    "mask_band": True,      # mask only the 128-wide diagonal band
    "bias_engine": "scalar",  # "vector" | "scalar"
    "spair": False,         # bf16 scores PSUM: TRN3-only, keep False on trn2
    "po_bf16": False,       # bf16 out-proj PSUM: TRN3-only
    "qkv_split": False,     # emit QKV chains as 2 half-units (smoother)
    "drain_order": "fillers",  # "fillers" | "pt": pt-first carries po units
                               # forward so the final batch stays fed
}

# dev-only A/B override, e.g. KCFG="vt=pe,pair=False"; harmless when unset
for _kv in __import__("os").environ.get("KCFG", "").split(","):
    if "=" in _kv:
        _k, _v = _kv.split("=", 1)
        CFG[_k] = {"True": True, "False": False}.get(_v, _v)


def _emit(tc, nc, xT, wqkvT, bqkv, woutT, outT, reps=1):
    G = reps * B  # global batch count
    ctx = ExitStack()
    with ctx:
        const = ctx.enter_context(tc.tile_pool(name="const", bufs=1))
        xpool = ctx.enter_context(tc.tile_pool(name="xpool", bufs=3))
        qkvpool = ctx.enter_context(tc.tile_pool(name="qkvpool", bufs=2))
        vpool = ctx.enter_context(tc.tile_pool(name="vpool", bufs=2))
        ppool = ctx.enter_context(
            tc.tile_pool(name="ppool", bufs=4 if CFG["spair"] else 8))
        ypool = ctx.enter_context(tc.tile_pool(name="ypool", bufs=3))
        opool = ctx.enter_context(tc.tile_pool(name="opool", bufs=2))
        spsum = ctx.enter_context(tc.tile_pool(name="spsum", bufs=2, space="PSUM"))
        accps = ctx.enter_context(tc.tile_pool(name="accps", bufs=1, space="PSUM"))
        qkps = ctx.enter_context(tc.tile_pool(name="qkps", bufs=2, space="PSUM"))

        # ---- constants ----
        # weights/bias first: the first QKV matmuls need them immediately;
        # identity/mask2 are Pool-engine builds that overlap the first
        # QKV matmul chain and aren't read until V-transpose / attention.
        w_sb = const.tile([128, NCT, F], BF16)      # wqkvT tiles: [c-tile][f]
        b_sb = const.tile([128, 3], FP32)
        for ft in range(3):
            nc.gpsimd.dma_start(b_sb[:, ft:ft + 1],
                                bqkv[ft * 128:(ft + 1) * 128].unsqueeze(1))
        x0_sb = xpool.tile([128, NCT, TQB], BF16, name="x_0_0", tag="x")
        for ct in range(NCT):
            # alternate HWDGE queues so the first QKV chain's operands land
            # at double the single-queue rate
            eng = nc.sync if ct % 2 == 0 else nc.scalar
            eng.dma_start(w_sb[:, ct, :], wqkvT[ct * 128:(ct + 1) * 128, :])
            eng.dma_start(x0_sb[:, ct, :],
                          xT[ct * 128:(ct + 1) * 128, 0:TQB])
        wo_sb = const.tile([128, C], BF16)          # woutT [cy, o]
        nc.sync.dma_start(wo_sb, woutT)
        if CFG["vt"] in ("pe", "pe2"):
            identity = const.tile([128, 128], BF16)
            make_identity(nc, identity)
        # 4 diagonal-block causal masks, each replicated for the 2 heads:
        # mask2[m][p, h*512 + q] = 1.0 if p <= q - 128*m else 0.0
        mask2 = []
        for m in range(4):
            mk = const.tile([128, 2 * TQB], BF16, name=f"mask2_{m}")
            nc.gpsimd.memset(mk, 1.0)
            for h in range(2):
                nc.gpsimd.affine_select(
                    out=mk[:, h * TQB:(h + 1) * TQB],
                    in_=mk[:, h * TQB:(h + 1) * TQB],
                    compare_op=ALU.is_ge,
                    fill=0.0,
                    base=-128 * m,
                    pattern=[[1, TQB]],
                    channel_multiplier=-1,
                )
            mask2.append(mk)

        x_tiles = {(0, 0): x0_sb}
        qkv_tiles = {}
        vab_tiles = {}

        def xdma(g, jj):
            if (g, jj) in x_tiles:
                return
            b = g % B
            x_sb = xpool.tile([128, NCT, TQB], BF16, name=f"x_{g}_{jj}",
                              tag="x")
            nc.sync.dma_start(
                x_sb,
                xT[:, (b * NJ + jj) * TQB:(b * NJ + jj + 1) * TQB].rearrange(
                    "(ct p) q -> p ct q", p=128))
            x_tiles[(g, jj)] = x_sb

        qkv_ps = {}

        def qkv_unit(g, jj, ft, half=None):
            """half=None: full 8-ct chain; half=0/1: first/second 4 cts.
            The second half evacuates PSUM with the bias add."""
            if g not in qkv_tiles:
                qkv_tiles[g] = qkvpool.tile([128, 3, T], BF16,
                                            name=f"qkv_{g}", tag="qkv")
            x_sb = x_tiles[(g, jj)]
            if half in (None, 0):
                ps = qkps.tile([128, TQB], FP32, name=f"qkvps_{g}_{jj}_{ft}",
                               tag="qk")
                qkv_ps[(g, jj, ft)] = ps
            else:
                ps = qkv_ps.pop((g, jj, ft))
            cts = range(NCT) if half is None else \
                range(4 * half, 4 * half + 4)
            for ct in cts:
                nc.tensor.matmul(ps,
                                 lhsT=w_sb[:, ct, ft * 128:(ft + 1) * 128],
                                 rhs=x_sb[:, ct, :],
                                 start=(ct == 0), stop=(ct == NCT - 1))
            if half == 0:
                return
            dst = qkv_tiles[g][:, ft, jj * TQB:(jj + 1) * TQB]
            if CFG["bias_engine"] == "vector":
                nc.vector.tensor_scalar(dst, ps, b_sb[:, ft:ft + 1], None,
                                        ALU.add)
            else:
                nc.scalar.activation(dst, ps, AF.Identity,
                                     bias=b_sb[:, ft:ft + 1])

        def vab_init(g):
            vab = vpool.tile([128, NKT, 256], BF16, name=f"vab_{g}",
                             tag="vab")
            vab_tiles[g] = vab
            # ones halves for the rowsum columns of the PV stationaries
            nc.gpsimd.memset(vab[:, :, 64:192], 1.0)

        def vt_unit4(g, jj):
            # all 4 key-tile transposes of a jj block back-to-back (PE
            # pipelines them) into one half-bank PSUM scratch, then two
            # strided group copies into vab.
            vab = vab_tiles[g]
            qkv_g = qkv_tiles[g]
            pvt = qkps.tile([128, 512], BF16, name=f"vt4_{g}_{jj}", tag="qk")
            for tt in range(4):
                i = 4 * jj + tt
                nc.tensor.transpose(pvt[:, tt * 128:(tt + 1) * 128],
                                    qkv_g[:, 2, i * 128:(i + 1) * 128],
                                    identity)
            p4 = pvt[:, :].rearrange("p (t c) -> p t c", t=4)
            nc.vector.tensor_copy(vab[:, 4 * jj:4 * jj + 4, 0:64],
                                  p4[:, :, 0:64])
            nc.vector.tensor_copy(vab[:, 4 * jj:4 * jj + 4, 192:256],
                                  p4[:, :, 64:128])

        def vt_unit(g, i):
            vab = vab_tiles[g]
            qkv_g = qkv_tiles[g]
            if CFG["vt"] == "dma":
                # XBAR transpose DMA: [64 vchan, 128 tok] -> [128 tok, 64]
                nc.sync.dma_start(vab[:, i, 0:64],
                                  qkv_g[0:64, 2, i * 128:(i + 1) * 128],
                                  transpose=True)
                nc.sync.dma_start(vab[:, i, 192:256],
                                  qkv_g[64:128, 2, i * 128:(i + 1) * 128],
                                  transpose=True)
            else:
                pvt = qkps.tile([128, 2 * TQB], BF16, name=f"vt_{g}_{i}",
                                tag="qk")
                nc.tensor.transpose(pvt[:, 0:128],
                                    qkv_g[:, 2, i * 128:(i + 1) * 128],
                                    identity)
                nc.vector.tensor_copy(vab[:, i, 0:64], pvt[:, 0:64])
                nc.vector.tensor_copy(vab[:, i, 192:256], pvt[:, 64:128])

        def pt_units(g):
            """Producer units (x DMA, QKV matmuls, V transposes) for batch g,
            in dependency order."""
            units = [lambda g=g: vab_init(g),
                     lambda g=g: xdma(g, 0), lambda g=g: xdma(g, 1)]
            for jj in range(NJ):
                if jj >= 1 and jj + 1 < NJ:
                    units.append(lambda g=g, jj=jj + 1: xdma(g, jj))
                for ft in range(3):
                    if CFG["qkv_split"]:
                        units.append(
                            lambda g=g, jj=jj, ft=ft: qkv_unit(g, jj, ft, 0))
                        units.append(
                            lambda g=g, jj=jj, ft=ft: qkv_unit(g, jj, ft, 1))
                    else:
                        units.append(
                            lambda g=g, jj=jj, ft=ft: qkv_unit(g, jj, ft))
                if CFG["vt"] == "pe2":
                    units.append(lambda g=g, jj=jj: vt_unit4(g, jj))
                else:
                    for i in range(4 * jj, 4 * jj + 4):
                        units.append(lambda g=g, i=i: vt_unit(g, i))
            return units

        def scores_pair(g, ta, tb):
            """Scores + exp + mask for two tiles (ta, tb) sharing one 2-bank
            bf16 PSUM pair tile: bank h holds head h of both tiles, so the
            row-tiled head matmuls land on different banks (required), and a
            full-full pair exps in ONE contiguous 2048-wide call.
            Returns {tile: (pp, u)} views for pv consumption."""
            qkv_g = qkv_tiles[g]
            ps = spsum.tile([128, 2, 2, TQB], BF16,
                            name=f"s_{g}_{ta[0]}_{ta[1]}", tag="s")
            pp = ppool.tile([128, 2, 2, TQB], BF16,
                            name=f"p_{g}_{ta[0]}_{ta[1]}", tag="p")
            q0s = []
            for u, (j, i) in enumerate((ta, tb)):
                m = i - 4 * j
                q0 = 128 * m if m > 0 else 0
                q0s.append(q0)
                for h in range(2):
                    nc.tensor.matmul(
                        ps[:, h, u, q0:],
                        lhsT=qkv_g[h * 64:(h + 1) * 64, 1,
                                   i * 128:(i + 1) * 128],
                        rhs=qkv_g[h * 64:(h + 1) * 64, 0,
                                  j * TQB + q0:(j + 1) * TQB],
                        start=True, stop=True,
                        tile_position=(h * 64, 0))
            if q0s == [0, 0]:
                # both untrimmed: one contiguous 2048-wide exp
                nc.scalar.activation(pp[:, :, :, :], ps[:, :, :, :],
                                     AF.Exp, scale=float(SCALE))
            else:
                for u, (j, i) in enumerate((ta, tb)):
                    q0 = q0s[u]
                    nc.scalar.activation(pp[:, :, u, q0:], ps[:, :, u, q0:],
                                         AF.Exp, scale=float(SCALE))
            for u, (j, i) in enumerate((ta, tb)):
                m = i - 4 * j
                if m >= 0:
                    q0 = q0s[u]
                    mk3 = mask2[m][:, :].rearrange("p (h q) -> p h q", h=2)
                    nc.vector.tensor_mul(pp[:, :, u, q0:q0 + 128],
                                         pp[:, :, u, q0:q0 + 128],
                                         mk3[:, :, q0:q0 + 128])
            return {ta: (pp, 0), tb: (pp, 1)}

        def scores_unit(g, j, i):
            """Score matmuls + exp + causal mask for tile i of block j.
            Diagonal tiles (m = i-4j >= 0) are trimmed to the unmasked
            q-range [128m, 512) -- the trimmed-away region is never read."""
            qkv_g = qkv_tiles[g]
            m = i - 4 * j
            q0 = 128 * m if m > 0 else 0
            ps2 = spsum.tile([128, 2 * TQB], FP32, name=f"s_{g}_{j}_{i}",
                             tag="s")
            for h in range(2):
                nc.tensor.matmul(
                    ps2[:, h * TQB + q0:(h + 1) * TQB],
                    lhsT=qkv_g[h * 64:(h + 1) * 64, 1, i * 128:(i + 1) * 128],
                    rhs=qkv_g[h * 64:(h + 1) * 64, 0,
                              j * TQB + q0:(j + 1) * TQB],
                    start=True, stop=True,
                    tile_position=(h * 64, 0))
            p_sb = ppool.tile([128, 2 * TQB], BF16, name=f"p_{g}_{j}_{i}",
                              tag="p")
            # 3-D views [128, head, q] for strided per-head slicing
            ps3 = ps2[:, :].rearrange("p (h q) -> p h q", h=2)
            pb3 = p_sb[:, :].rearrange("p (h q) -> p h q", h=2)
            if q0:
                if CFG["exp_merge"]:
                    nc.scalar.activation(pb3[:, :, q0:], ps3[:, :, q0:],
                                         AF.Exp, scale=float(SCALE))
                else:
                    for h in range(2):
                        nc.scalar.activation(
                            p_sb[:, h * TQB + q0:(h + 1) * TQB],
                            ps2[:, h * TQB + q0:(h + 1) * TQB],
                            AF.Exp, scale=float(SCALE))
            else:
                nc.scalar.activation(p_sb, ps2, AF.Exp, scale=float(SCALE))
            if m >= 0:
                mk3 = mask2[m][:, :].rearrange("p (h q) -> p h q", h=2)
                if CFG["mask_band"]:
                    # only the diagonal 128x128 sub-block needs masking:
                    # cols >= q0+128 of the live range are fully unmasked
                    meng = nc.gpsimd if CFG["mask_engine"] == "gpsimd" \
                        else nc.vector
                    meng.tensor_mul(pb3[:, :, q0:q0 + 128],
                                    pb3[:, :, q0:q0 + 128],
                                    mk3[:, :, q0:q0 + 128])
                elif q0:
                    for h in range(2):
                        nc.vector.tensor_mul(
                            p_sb[:, h * TQB + q0:(h + 1) * TQB],
                            p_sb[:, h * TQB + q0:(h + 1) * TQB],
                            mask2[m][:, h * TQB + q0:(h + 1) * TQB])
                elif CFG["mask_engine"] == "split":
                    nc.gpsimd.tensor_mul(p_sb, p_sb, mask2[m])
                else:
                    nc.vector.tensor_mul(p_sb, p_sb, mask2[m])
            return p_sb

        def pv_unit(g, j, i, ntk, p_ref, pyA, pyB):
            vab = vab_tiles[g]
            m = i - 4 * j
            q0 = 128 * m if m > 0 else 0
            first, last = (i == 0), (i == ntk - 1)
            if CFG["spair"]:
                pp, u = p_ref
                rh0, rh1 = pp[:, 0, u, q0:], pp[:, 1, u, q0:]
            else:
                p_sb = p_ref
                rh0 = p_sb[:, q0:TQB]
                rh1 = p_sb[:, TQB + q0:2 * TQB]
            nc.tensor.matmul(pyA[:, q0:TQB], lhsT=vab[:, i, 0:128],
                             rhs=rh0, start=first, stop=last)
            nc.tensor.matmul(pyB[:, q0:TQB], lhsT=vab[:, i, 128:256],
                             rhs=rh1, start=first, stop=last)

        def norm(g, j, pyA, pyB):
            # pyA = [Y_h0 (0:64); r_h0 (64:128)], pyB = [r_h1 (0:64); Y_h1].
            y_sb = ypool.tile([128, TQB], BF16, name=f"y_{g}_{j}", tag="y")
            rc = ypool.tile([128, TQB], FP32, name=f"rc_{g}_{j}", tag="rc")
            if CFG["norm"] == "psum2":
                # recips run straight off the PSUM rowsums with equal in/out
                # partition offsets (the approx custom uop NaNs only when
                # offsets differ); the multiplies then read rc crosswise.
                nc.vector.reciprocal_approx_fast(rc[0:64, :], pyB[0:64, :])
                nc.vector.reciprocal_approx_fast(rc[64:128, :],
                                                 pyA[64:128, :])
                nc.vector.tensor_mul(y_sb[0:64, :], pyA[0:64, :],
                                     rc[64:128, :])
                nc.vector.tensor_mul(y_sb[64:128, :], pyB[64:128, :],
                                     rc[0:64, :])
            else:
                # two partition-offset-shifted copies gather the rowsums,
                # then a full-width zero-offset reciprocal + two multiplies.
                rs = ypool.tile([128, TQB], FP32, name=f"rs_{g}_{j}",
                                tag="rs")
                nc.vector.tensor_copy(rs[0:64, :], pyA[64:128, :])
                nc.vector.tensor_copy(rs[64:128, :], pyB[0:64, :])
                nc.vector.reciprocal_approx_fast(rc, rs)
                nc.vector.tensor_mul(y_sb[0:64, :], pyA[0:64, :],
                                     rc[0:64, :])
                nc.vector.tensor_mul(y_sb[64:128, :], pyB[64:128, :],
                                     rc[64:128, :])
            return y_sb

        def outproj_units(g, j, y_sb, last=False):
            b = g % B
            o_sb = opool.tile([128, NCT, TQB], BF16, name=f"o_{g}_{j}",
                              tag="o")

            def po_unit(ot, o_sb=o_sb, y_sb=y_sb, g=g, j=j, b=b, last=last):
                po = qkps.tile([128, TQB],
                               BF16 if CFG["po_bf16"] else FP32,
                               name=f"po_{g}_{j}_{ot}", tag="qk")
                nc.tensor.matmul(po, lhsT=wo_sb[:, ot * 128:(ot + 1) * 128],
                                 rhs=y_sb, start=True, stop=True)
                if last and ot % 2 == 1:
                    # drain-phase: split evacuation across ACT+DVE so the
                    # final block's tail halves
                    nc.scalar.activation(o_sb[:, ot, :], po, AF.Identity)
                else:
                    nc.vector.tensor_copy(o_sb[:, ot, :], po)
                if ot % 2 == 1:
                    nc.gpsimd.dma_start(
                        outT[(ot - 1) * 128:(ot + 1) * 128,
                             b * T + j * TQB:b * T + (j + 1) * TQB]
                        .rearrange("(o p) q -> p o q", p=128),
                        o_sb[:, ot - 1:ot + 1, :])

            return [lambda ot=ot: po_unit(ot) for ot in range(NCT)]

        # ---- main schedule ----
        fillers = deque()  # carried out-proj units (safe across batches)
        for u in pt_units(0):
            u()
        for g in range(G):
            pt = deque(pt_units(g + 1)) if g + 1 < G else deque()

            def drain_one(pt=pt):
                first, second = ((pt, fillers)
                                 if CFG["drain_order"] == "pt"
                                 else (fillers, pt))
                if first:
                    first.popleft()()
                elif second:
                    second.popleft()()
            tiles_left = NJ * (NJ + 1) * 2  # 40 attention tiles per batch
            # flat tile stream for the whole batch with scores running two
            # tiles ahead of PV (across block boundaries): exp/mask of tile
            # t completes while PE streams scores of t+1/t+2, and the norm
            # chain of block j drains while scores of block j+1 issue.
            seq = [(j, i) for j in range(NJ) for i in range(4 * (j + 1))]
            pys = {}
            p_tiles = {}

            def ensure_py(j):
                if j not in pys:
                    pys[j] = (accps.tile([128, TQB], FP32, name=f"pyA_{g}_{j}",
                                         tag="pyA"),
                              accps.tile([128, TQB], FP32, name=f"pyB_{g}_{j}",
                                         tag="pyB"))
                return pys[j]

            if CFG["spair"]:
                LOOK = 4   # two pair tiles in flight ahead of PV
                for k in range(0, min(LOOK, len(seq)), 2):
                    p_tiles.update(scores_pair(g, seq[k], seq[k + 1]))
            else:
                LOOK = 2
                for k in range(min(LOOK, len(seq))):
                    p_tiles[seq[k]] = scores_unit(g, *seq[k])
            STRIDE = 2 if (CFG["pair"] or CFG["spair"]) else 1
            for t0 in range(0, len(seq), STRIDE):
                # pre-drain: half the step's filler quota ahead of the
                # scores group covers the exp latency that would otherwise
                # stall the PE at the head of the 64-mode score matmuls
                pre_drained = 0
                if CFG["interleave"] and tiles_left > STRIDE:
                    n = len(fillers) + len(pt)
                    quota = -(-n * STRIDE // max(tiles_left, 1))
                    pre_drained = min((quota + 1) // 2, n)
                    for _ in range(pre_drained):
                        drain_one()
                # scores for the next pair together: one 64x128-mode group
                # per step instead of two (array retiling drains the PE)
                if CFG["spair"]:
                    u = t0 + LOOK
                    if u + 1 < len(seq):
                        p_tiles.update(scores_pair(g, seq[u], seq[u + 1]))
                else:
                    for u in range(t0 + LOOK,
                                   min(t0 + LOOK + STRIDE, len(seq))):
                        p_tiles[seq[u]] = scores_unit(g, *seq[u])
                drain = 0
                for t in range(t0, min(t0 + STRIDE, len(seq))):
                    j, i = seq[t]
                    ntk = 4 * (j + 1)
                    pyA, pyB = ensure_py(j)
                    pv_unit(g, j, i, ntk, p_tiles.pop((j, i)), pyA, pyB)
                    tiles_left -= 1
                    if i == ntk - 1:
                        y_sb = norm(g, j, pyA, pyB)
                        last = (g == G - 1) and (j == NJ - 1)
                        fillers.extend(outproj_units(g, j, y_sb, last=last))
                        del pys[j]
                    elif CFG["interleave"]:
                        # drain fillers (out-proj first, then next-batch
                        # producers) at a rate that finishes by batch end;
                        # the last PV of a block is followed by norm() so
                        # the recips hit the DVE queue first (they gate the
                        # next block's PSUM), extra drains early cover the
                        # gap.
                        n = len(fillers) + len(pt)
                        k = -(-n // max(tiles_left, 1)) if tiles_left else n
                        if i < 2:
                            k += 1
                        drain += k
                drain = max(drain - pre_drained, 0)
                n = len(fillers) + len(pt)
                for _ in range(min(drain, n)):
                    drain_one()
            if not CFG["interleave"]:
                while fillers:
                    fillers.popleft()()
            # next-batch producers must be fully emitted before the next
            # batch's attention reads them (engines execute in program order)
            while pt:
                if fillers:
                    fillers.popleft()()
                pt.popleft()()
        while fillers:
            fillers.popleft()()


def build(reps=1):
    nc = bacc.Bacc()
    xT = nc.dram_tensor("xT", [C, BT], BF16, kind="ExternalInput")
    wqkvT = nc.dram_tensor("wqkvT", [C, F], BF16, kind="ExternalInput")
    bqkv = nc.dram_tensor("bqkv", [F], FP32, kind="ExternalInput")
    woutT = nc.dram_tensor("woutT", [CPC, C], BF16, kind="ExternalInput")
    outT = nc.dram_tensor("outT", [C, BT], BF16, kind="ExternalOutput")
    with tile.TileContext(nc) as tc:
        _emit(tc, nc, xT.ap(), wqkvT.ap(), bqkv.ap(), woutT.ap(), outT.ap(),
              reps=reps)
    nc.compile()
    return nc


def make_in_maps(x, attention_mask, Wqkv, bqkv, Wout):
    # attention_mask is all-ones per the problem spec (fill: ones) -- the
    # kernel bakes in causal-only masking.
    xT = np.ascontiguousarray(
        x.reshape(BT, C).T).astype(NPBF16)
    in_maps = []
    for c in range(NCORES):
        rows = np.r_[c * CPC:(c + 1) * CPC,
                     C + c * CPC:C + (c + 1) * CPC,
                     2 * C + c * CPC:2 * C + (c + 1) * CPC]
        wqkvT_c = np.ascontiguousarray(Wqkv[rows, :].T).astype(NPBF16)
        b_c = np.ascontiguousarray(bqkv[rows].astype(np.float32, copy=False))
        woutT_c = np.ascontiguousarray(
            Wout[:, c * CPC:(c + 1) * CPC].T).astype(NPBF16)
        in_maps.append({"xT": xT, "wqkvT": wqkvT_c, "bqkv": b_c,
                        "woutT": woutT_c})
    return in_maps


def kernel(x, attention_mask, Wqkv, bqkv, Wout, _trace=False):
    x = np.asarray(x)
    attention_mask = np.asarray(attention_mask)
    Wqkv = np.asarray(Wqkv)
    bqkv = np.asarray(bqkv)
    Wout = np.asarray(Wout)
    if "nc" not in _cached:
        _cached["nc"] = build()
    nc = _cached["nc"]
    in_maps = make_in_maps(x, attention_mask, Wqkv, bqkv, Wout)
    res = bass_utils.run_bass_kernel_spmd(
        nc, in_maps, core_ids=list(range(NCORES)), trace=_trace)
    acc = res.results[0]["outT"].astype(np.float32)
    for r in res.results[1:]:
        acc += r["outT"].astype(np.float32)
    out = np.ascontiguousarray(acc.T).reshape(B, T, C).astype(np.float32)
    if _trace:
        _cached["last_result"] = res
    return out



# revision 37
# speedup vs baseline: 1.1128x; 1.1128x over previous
"""Causal self-attention (B=4, T=2048, C=1024, H=16) on 8 trn2 NeuronCores.

Head-parallel tensor parallelism: each core owns 2 of 16 heads (its 384 rows
of Wqkv, its 128 columns of Wout); the 8 partial [C, B*T] outputs are summed
on the host (the all-reduce). HW-measured (NTFF) ~336us vs ~521us baseline.

Design:
 - bf16 datapath on SBUF (bf16 matmul = 1 cycle/row, same as fp32r, but DMA
   bytes and DVE element costs halve); PSUM/bias/softmax-denominator fp32.
 - attention_mask is all-ones per the problem spec (fill: ones) -> causal
   mask only, no pad machinery.
 - scores held transposed (S^T [tk, tq]) so the softmax denominator is a
   partition-dim sum that rides the PV matmul for free via an augmented
   stationary [v | ones]; normalize = 2 copies + reciprocal_approx_fast +
   2 multiplies on DVE (full reciprocal() costs 3.35us/call on HW; the
   approx custom op NaNs with partition-offset operands, hence the copies).
 - causal diagonal tiles trimmed: score matmuls, exp, and mask only cover
   the live q-range [128m, 512); PV reads the same range.
 - software pipelining: scores run ahead of PV; QKV + V-transpose units of
   batch b+1 and out-proj units of the previous q-block are interleaved
   between attention tiles so TensorE (roofline engine, ~275us busy) never
   starves; batched DMAs split across SP (x in), gpsimd/SWDGE (out).

Layouts (matmul contractions always on SBUF partitions):
  xT    [C, B*T]      x transposed on host, bf16
  qkv   [128, 3, T]   per batch; rows = 2 heads x 64 dims
  vAB   [128, i, 256] per key-tile: v_h0 | ones | ones | v_h1
  pyA   [Y_h0; r_h0]  PV psum (rows 64:128 = replicated rowsum)
  pyB   [r_h1; Y_h1]
  outT  [C, B*T]      partial output, bf16, summed across cores on host
"""

import numpy as np
from collections import deque
from contextlib import ExitStack

import concourse.bass as bass
import concourse.bacc as bacc
import concourse.mybir as mybir
import concourse.tile as tile
from concourse import bass_utils
from concourse.masks import make_identity

B, T, C = 4, 2048, 1024
H, D = 16, 64
NCORES = 8
HPC = H // NCORES            # heads per core = 2
CPC = HPC * D                # y-channels per core = 128
BT = B * T                   # 8192
F = 3 * CPC                  # qkv rows per core = 384
TQB = 512                    # tq block (matmul free dim)
NJ = T // TQB                # 4 tq blocks per batch
NKT = T // 128               # 16 tk tiles per batch
NCT = C // 128               # 8 contraction tiles for projections
FP32 = mybir.dt.float32
BF16 = mybir.dt.bfloat16
AF = mybir.ActivationFunctionType
ALU = mybir.AluOpType
SCALE = 1.0 / np.sqrt(D)
NPBF16 = mybir.dt.np(BF16)

_cached = {}

CFG = {
    "norm": "copies",       # "psum2" (recips straight off PSUM) | "copies"
    "mask_engine": "vector",  # "vector" | "split" (m<2 on pool)
    "interleave": True,
    "vt": "pe2",            # "pe2" (grouped PE transpose) | "dma" | "pe"
    "pair": True,           # process attention tiles two at a time
    "exp_merge": True,      # one strided exp call for trimmed diag tiles
    "mask_band": True,      # mask only the 128-wide diagonal band
    "bias_engine": "scalar",  # "vector" | "scalar"
    "spair": False,         # bf16 scores PSUM: TRN3-only, keep False on trn2
    "po_bf16": False,       # bf16 out-proj PSUM: TRN3-only
    "qkv_split": False,     # emit QKV chains as 2 half-units (smoother)
    "drain_order": "fillers",  # "fillers" | "pt": pt-first carries po units
                               # forward so the final batch stays fed
    "pre_drain": False,     # fillers ahead of scores group (measured worse:
                            # delaying scores delays exp, the long pole)
}

# dev-only A/B override, e.g. KCFG="vt=pe,pair=False"; harmless when unset
for _kv in __import__("os").environ.get("KCFG", "").split(","):
    if "=" in _kv:
        _k, _v = _kv.split("=", 1)
        CFG[_k] = {"True": True, "False": False}.get(_v, _v)


def _emit(tc, nc, xT, wqkvT, bqkv, woutT, outT, reps=1):
    G = reps * B  # global batch count
    ctx = ExitStack()
    with ctx:
        const = ctx.enter_context(tc.tile_pool(name="const", bufs=1))
        xpool = ctx.enter_context(tc.tile_pool(name="xpool", bufs=3))
        qkvpool = ctx.enter_context(tc.tile_pool(name="qkvpool", bufs=2))
        vpool = ctx.enter_context(tc.tile_pool(name="vpool", bufs=2))
        ppool = ctx.enter_context(
            tc.tile_pool(name="ppool", bufs=4 if CFG["spair"] else 8))
        ypool = ctx.enter_context(tc.tile_pool(name="ypool", bufs=3))
        opool = ctx.enter_context(tc.tile_pool(name="opool", bufs=2))
        spsum = ctx.enter_context(tc.tile_pool(name="spsum", bufs=2, space="PSUM"))
        accps = ctx.enter_context(tc.tile_pool(name="accps", bufs=1, space="PSUM"))
        qkps = ctx.enter_context(tc.tile_pool(name="qkps", bufs=2, space="PSUM"))

        # ---- constants ----
        # weights/bias first: the first QKV matmuls need them immediately;
        # identity/mask2 are Pool-engine builds that overlap the first
        # QKV matmul chain and aren't read until V-transpose / attention.
        w_sb = const.tile([128, NCT, F], BF16)      # wqkvT tiles: [c-tile][f]
        b_sb = const.tile([128, 3], FP32)
        for ft in range(3):
            nc.gpsimd.dma_start(b_sb[:, ft:ft + 1],
                                bqkv[ft * 128:(ft + 1) * 128].unsqueeze(1))
        x0_sb = xpool.tile([128, NCT, TQB], BF16, name="x_0_0", tag="x")
        for ct in range(NCT):
            # alternate HWDGE queues so the first QKV chain's operands land
            # at double the single-queue rate
            eng = nc.sync if ct % 2 == 0 else nc.scalar
            eng.dma_start(w_sb[:, ct, :], wqkvT[ct * 128:(ct + 1) * 128, :])
            eng.dma_start(x0_sb[:, ct, :],
                          xT[ct * 128:(ct + 1) * 128, 0:TQB])
        wo_sb = const.tile([128, C], BF16)          # woutT [cy, o]
        nc.sync.dma_start(wo_sb, woutT)
        if CFG["vt"] in ("pe", "pe2"):
            identity = const.tile([128, 128], BF16)
            make_identity(nc, identity)
        # 4 diagonal-block causal masks, each replicated for the 2 heads:
        # mask2[m][p, h*512 + q] = 1.0 if p <= q - 128*m else 0.0
        mask2 = []
        for m in range(4):
            mk = const.tile([128, 2 * TQB], BF16, name=f"mask2_{m}")
            nc.gpsimd.memset(mk, 1.0)
            for h in range(2):
                nc.gpsimd.affine_select(
                    out=mk[:, h * TQB:(h + 1) * TQB],
                    in_=mk[:, h * TQB:(h + 1) * TQB],
                    compare_op=ALU.is_ge,
                    fill=0.0,
                    base=-128 * m,
                    pattern=[[1, TQB]],
                    channel_multiplier=-1,
                )
            mask2.append(mk)

        x_tiles = {(0, 0): x0_sb}
        qkv_tiles = {}
        vab_tiles = {}

        def xdma(g, jj):
            if (g, jj) in x_tiles:
                return
            b = g % B
            x_sb = xpool.tile([128, NCT, TQB], BF16, name=f"x_{g}_{jj}",
                              tag="x")
            nc.sync.dma_start(
                x_sb,
                xT[:, (b * NJ + jj) * TQB:(b * NJ + jj + 1) * TQB].rearrange(
                    "(ct p) q -> p ct q", p=128))
            x_tiles[(g, jj)] = x_sb

        qkv_ps = {}

        def qkv_unit(g, jj, ft, half=None):
            """half=None: full 8-ct chain; half=0/1: first/second 4 cts.
            The second half evacuates PSUM with the bias add."""
            if g not in qkv_tiles:
                qkv_tiles[g] = qkvpool.tile([128, 3, T], BF16,
                                            name=f"qkv_{g}", tag="qkv")
            x_sb = x_tiles[(g, jj)]
            if half in (None, 0):
                ps = qkps.tile([128, TQB], FP32, name=f"qkvps_{g}_{jj}_{ft}",
                               tag="qk")
                qkv_ps[(g, jj, ft)] = ps
            else:
                ps = qkv_ps.pop((g, jj, ft))
            cts = range(NCT) if half is None else \
                range(4 * half, 4 * half + 4)
            for ct in cts:
                nc.tensor.matmul(ps,
                                 lhsT=w_sb[:, ct, ft * 128:(ft + 1) * 128],
                                 rhs=x_sb[:, ct, :],
                                 start=(ct == 0), stop=(ct == NCT - 1))
            if half == 0:
                return
            dst = qkv_tiles[g][:, ft, jj * TQB:(jj + 1) * TQB]
            if CFG["bias_engine"] == "vector":
                nc.vector.tensor_scalar(dst, ps, b_sb[:, ft:ft + 1], None,
                                        ALU.add)
            else:
                nc.scalar.activation(dst, ps, AF.Identity,
                                     bias=b_sb[:, ft:ft + 1])

        def vab_init(g):
            vab = vpool.tile([128, NKT, 256], BF16, name=f"vab_{g}",
                             tag="vab")
            vab_tiles[g] = vab
            # ones halves for the rowsum columns of the PV stationaries
            nc.gpsimd.memset(vab[:, :, 64:192], 1.0)

        def vt_unit4(g, jj):
            # all 4 key-tile transposes of a jj block back-to-back (PE
            # pipelines them) into one half-bank PSUM scratch, then two
            # strided group copies into vab.
            vab = vab_tiles[g]
            qkv_g = qkv_tiles[g]
            pvt = qkps.tile([128, 512], BF16, name=f"vt4_{g}_{jj}", tag="qk")
            for tt in range(4):
                i = 4 * jj + tt
                nc.tensor.transpose(pvt[:, tt * 128:(tt + 1) * 128],
                                    qkv_g[:, 2, i * 128:(i + 1) * 128],
                                    identity)
            p4 = pvt[:, :].rearrange("p (t c) -> p t c", t=4)
            nc.vector.tensor_copy(vab[:, 4 * jj:4 * jj + 4, 0:64],
                                  p4[:, :, 0:64])
            nc.vector.tensor_copy(vab[:, 4 * jj:4 * jj + 4, 192:256],
                                  p4[:, :, 64:128])

        def vt_unit(g, i):
            vab = vab_tiles[g]
            qkv_g = qkv_tiles[g]
            if CFG["vt"] == "dma":
                # XBAR transpose DMA: [64 vchan, 128 tok] -> [128 tok, 64]
                nc.sync.dma_start(vab[:, i, 0:64],
                                  qkv_g[0:64, 2, i * 128:(i + 1) * 128],
                                  transpose=True)
                nc.sync.dma_start(vab[:, i, 192:256],
                                  qkv_g[64:128, 2, i * 128:(i + 1) * 128],
                                  transpose=True)
            else:
                pvt = qkps.tile([128, 2 * TQB], BF16, name=f"vt_{g}_{i}",
                                tag="qk")
                nc.tensor.transpose(pvt[:, 0:128],
                                    qkv_g[:, 2, i * 128:(i + 1) * 128],
                                    identity)
                nc.vector.tensor_copy(vab[:, i, 0:64], pvt[:, 0:64])
                nc.vector.tensor_copy(vab[:, i, 192:256], pvt[:, 64:128])

        def pt_units(g):
            """Producer units (x DMA, QKV matmuls, V transposes) for batch g,
            in dependency order."""
            units = [lambda g=g: vab_init(g),
                     lambda g=g: xdma(g, 0), lambda g=g: xdma(g, 1)]
            for jj in range(NJ):
                if jj >= 1 and jj + 1 < NJ:
                    units.append(lambda g=g, jj=jj + 1: xdma(g, jj))
                for ft in range(3):
                    if CFG["qkv_split"]:
                        units.append(
                            lambda g=g, jj=jj, ft=ft: qkv_unit(g, jj, ft, 0))
                        units.append(
                            lambda g=g, jj=jj, ft=ft: qkv_unit(g, jj, ft, 1))
                    else:
                        units.append(
                            lambda g=g, jj=jj, ft=ft: qkv_unit(g, jj, ft))
                if CFG["vt"] == "pe2":
                    units.append(lambda g=g, jj=jj: vt_unit4(g, jj))
                else:
                    for i in range(4 * jj, 4 * jj + 4):
                        units.append(lambda g=g, i=i: vt_unit(g, i))
            return units

        def scores_pair(g, ta, tb):
            """Scores + exp + mask for two tiles (ta, tb) sharing one 2-bank
            bf16 PSUM pair tile: bank h holds head h of both tiles, so the
            row-tiled head matmuls land on different banks (required), and a
            full-full pair exps in ONE contiguous 2048-wide call.
            Returns {tile: (pp, u)} views for pv consumption."""
            qkv_g = qkv_tiles[g]
            ps = spsum.tile([128, 2, 2, TQB], BF16,
                            name=f"s_{g}_{ta[0]}_{ta[1]}", tag="s")
            pp = ppool.tile([128, 2, 2, TQB], BF16,
                            name=f"p_{g}_{ta[0]}_{ta[1]}", tag="p")
            q0s = []
            for u, (j, i) in enumerate((ta, tb)):
                m = i - 4 * j
                q0 = 128 * m if m > 0 else 0
                q0s.append(q0)
                for h in range(2):
                    nc.tensor.matmul(
                        ps[:, h, u, q0:],
                        lhsT=qkv_g[h * 64:(h + 1) * 64, 1,
                                   i * 128:(i + 1) * 128],
                        rhs=qkv_g[h * 64:(h + 1) * 64, 0,
                                  j * TQB + q0:(j + 1) * TQB],
                        start=True, stop=True,
                        tile_position=(h * 64, 0))
            if q0s == [0, 0]:
                # both untrimmed: one contiguous 2048-wide exp
                nc.scalar.activation(pp[:, :, :, :], ps[:, :, :, :],
                                     AF.Exp, scale=float(SCALE))
            else:
                for u, (j, i) in enumerate((ta, tb)):
                    q0 = q0s[u]
                    nc.scalar.activation(pp[:, :, u, q0:], ps[:, :, u, q0:],
                                         AF.Exp, scale=float(SCALE))
            for u, (j, i) in enumerate((ta, tb)):
                m = i - 4 * j
                if m >= 0:
                    q0 = q0s[u]
                    mk3 = mask2[m][:, :].rearrange("p (h q) -> p h q", h=2)
                    nc.vector.tensor_mul(pp[:, :, u, q0:q0 + 128],
                                         pp[:, :, u, q0:q0 + 128],
                                         mk3[:, :, q0:q0 + 128])
            return {ta: (pp, 0), tb: (pp, 1)}

        def scores_unit(g, j, i):
            """Score matmuls + exp + causal mask for tile i of block j.
            Diagonal tiles (m = i-4j >= 0) are trimmed to the unmasked
            q-range [128m, 512) -- the trimmed-away region is never read."""
            qkv_g = qkv_tiles[g]
            m = i - 4 * j
            q0 = 128 * m if m > 0 else 0
            ps2 = spsum.tile([128, 2 * TQB], FP32, name=f"s_{g}_{j}_{i}",
                             tag="s")
            for h in range(2):
                nc.tensor.matmul(
                    ps2[:, h * TQB + q0:(h + 1) * TQB],
                    lhsT=qkv_g[h * 64:(h + 1) * 64, 1, i * 128:(i + 1) * 128],
                    rhs=qkv_g[h * 64:(h + 1) * 64, 0,
                              j * TQB + q0:(j + 1) * TQB],
                    start=True, stop=True,
                    tile_position=(h * 64, 0))
            p_sb = ppool.tile([128, 2 * TQB], BF16, name=f"p_{g}_{j}_{i}",
                              tag="p")
            # 3-D views [128, head, q] for strided per-head slicing
            ps3 = ps2[:, :].rearrange("p (h q) -> p h q", h=2)
            pb3 = p_sb[:, :].rearrange("p (h q) -> p h q", h=2)
            if q0:
                if CFG["exp_merge"]:
                    nc.scalar.activation(pb3[:, :, q0:], ps3[:, :, q0:],
                                         AF.Exp, scale=float(SCALE))
                else:
                    for h in range(2):
                        nc.scalar.activation(
                            p_sb[:, h * TQB + q0:(h + 1) * TQB],
                            ps2[:, h * TQB + q0:(h + 1) * TQB],
                            AF.Exp, scale=float(SCALE))
            else:
                nc.scalar.activation(p_sb, ps2, AF.Exp, scale=float(SCALE))
            if m >= 0:
                mk3 = mask2[m][:, :].rearrange("p (h q) -> p h q", h=2)
                if CFG["mask_band"]:
                    # only the diagonal 128x128 sub-block needs masking:
                    # cols >= q0+128 of the live range are fully unmasked
                    meng = nc.gpsimd if CFG["mask_engine"] == "gpsimd" \
                        else nc.vector
                    meng.tensor_mul(pb3[:, :, q0:q0 + 128],
                                    pb3[:, :, q0:q0 + 128],
                                    mk3[:, :, q0:q0 + 128])
                elif q0:
                    for h in range(2):
                        nc.vector.tensor_mul(
                            p_sb[:, h * TQB + q0:(h + 1) * TQB],
                            p_sb[:, h * TQB + q0:(h + 1) * TQB],
                            mask2[m][:, h * TQB + q0:(h + 1) * TQB])
                elif CFG["mask_engine"] == "split":
                    nc.gpsimd.tensor_mul(p_sb, p_sb, mask2[m])
                else:
                    nc.vector.tensor_mul(p_sb, p_sb, mask2[m])
            return p_sb

        def pv_unit(g, j, i, ntk, p_ref, pyA, pyB):
            vab = vab_tiles[g]
            m = i - 4 * j
            q0 = 128 * m if m > 0 else 0
            first, last = (i == 0), (i == ntk - 1)
            if CFG["spair"]:
                pp, u = p_ref
                rh0, rh1 = pp[:, 0, u, q0:], pp[:, 1, u, q0:]
            else:
                p_sb = p_ref
                rh0 = p_sb[:, q0:TQB]
                rh1 = p_sb[:, TQB + q0:2 * TQB]
            nc.tensor.matmul(pyA[:, q0:TQB], lhsT=vab[:, i, 0:128],
                             rhs=rh0, start=first, stop=last)
            nc.tensor.matmul(pyB[:, q0:TQB], lhsT=vab[:, i, 128:256],
                             rhs=rh1, start=first, stop=last)

        def norm(g, j, pyA, pyB):
            # pyA = [Y_h0 (0:64); r_h0 (64:128)], pyB = [r_h1 (0:64); Y_h1].
            y_sb = ypool.tile([128, TQB], BF16, name=f"y_{g}_{j}", tag="y")
            rc = ypool.tile([128, TQB], FP32, name=f"rc_{g}_{j}", tag="rc")
            if CFG["norm"] == "psum2":
                # recips run straight off the PSUM rowsums with equal in/out
                # partition offsets (the approx custom uop NaNs only when
                # offsets differ); the multiplies then read rc crosswise.
                nc.vector.reciprocal_approx_fast(rc[0:64, :], pyB[0:64, :])
                nc.vector.reciprocal_approx_fast(rc[64:128, :],
                                                 pyA[64:128, :])
                nc.vector.tensor_mul(y_sb[0:64, :], pyA[0:64, :],
                                     rc[64:128, :])
                nc.vector.tensor_mul(y_sb[64:128, :], pyB[64:128, :],
                                     rc[0:64, :])
            else:
                # two partition-offset-shifted copies gather the rowsums,
                # then a full-width zero-offset reciprocal + two multiplies.
                rs = ypool.tile([128, TQB], FP32, name=f"rs_{g}_{j}",
                                tag="rs")
                nc.vector.tensor_copy(rs[0:64, :], pyA[64:128, :])
                nc.vector.tensor_copy(rs[64:128, :], pyB[0:64, :])
                nc.vector.reciprocal_approx_fast(rc, rs)
                nc.vector.tensor_mul(y_sb[0:64, :], pyA[0:64, :],
                                     rc[0:64, :])
                nc.vector.tensor_mul(y_sb[64:128, :], pyB[64:128, :],
                                     rc[64:128, :])
            return y_sb

        def outproj_units(g, j, y_sb, last=False):
            b = g % B
            o_sb = opool.tile([128, NCT, TQB], BF16, name=f"o_{g}_{j}",
                              tag="o")

            def po_unit(ot, o_sb=o_sb, y_sb=y_sb, g=g, j=j, b=b, last=last):
                po = qkps.tile([128, TQB],
                               BF16 if CFG["po_bf16"] else FP32,
                               name=f"po_{g}_{j}_{ot}", tag="qk")
                nc.tensor.matmul(po, lhsT=wo_sb[:, ot * 128:(ot + 1) * 128],
                                 rhs=y_sb, start=True, stop=True)
                if last and ot % 2 == 1:
                    # drain-phase: split evacuation across ACT+DVE so the
                    # final block's tail halves
                    nc.scalar.activation(o_sb[:, ot, :], po, AF.Identity)
                else:
                    nc.vector.tensor_copy(o_sb[:, ot, :], po)
                if ot % 2 == 1:
                    nc.gpsimd.dma_start(
                        outT[(ot - 1) * 128:(ot + 1) * 128,
                             b * T + j * TQB:b * T + (j + 1) * TQB]
                        .rearrange("(o p) q -> p o q", p=128),
                        o_sb[:, ot - 1:ot + 1, :])

            return [lambda ot=ot: po_unit(ot) for ot in range(NCT)]

        # ---- main schedule ----
        fillers = deque()  # carried out-proj units (safe across batches)
        for u in pt_units(0):
            u()
        for g in range(G):
            pt = deque(pt_units(g + 1)) if g + 1 < G else deque()

            def drain_one(pt=pt):
                first, second = ((pt, fillers)
                                 if CFG["drain_order"] == "pt"
                                 else (fillers, pt))
                if first:
                    first.popleft()()
                elif second:
                    second.popleft()()
            tiles_left = NJ * (NJ + 1) * 2  # 40 attention tiles per batch
            # flat tile stream for the whole batch with scores running two
            # tiles ahead of PV (across block boundaries): exp/mask of tile
            # t completes while PE streams scores of t+1/t+2, and the norm
            # chain of block j drains while scores of block j+1 issue.
            seq = [(j, i) for j in range(NJ) for i in range(4 * (j + 1))]
            pys = {}
            p_tiles = {}

            def ensure_py(j):
                if j not in pys:
                    pys[j] = (accps.tile([128, TQB], FP32, name=f"pyA_{g}_{j}",
                                         tag="pyA"),
                              accps.tile([128, TQB], FP32, name=f"pyB_{g}_{j}",
                                         tag="pyB"))
                return pys[j]

            if CFG["spair"]:
                LOOK = 4   # two pair tiles in flight ahead of PV
                for k in range(0, min(LOOK, len(seq)), 2):
                    p_tiles.update(scores_pair(g, seq[k], seq[k + 1]))
            else:
                LOOK = 2
                for k in range(min(LOOK, len(seq))):
                    p_tiles[seq[k]] = scores_unit(g, *seq[k])
            STRIDE = 2 if (CFG["pair"] or CFG["spair"]) else 1
            for t0 in range(0, len(seq), STRIDE):
                # pre-drain: half the step's filler quota ahead of the
                # scores group covers the exp latency that would otherwise
                # stall the PE at the head of the 64-mode score matmuls
                pre_drained = 0
                if CFG["pre_drain"] and CFG["interleave"] \
                        and tiles_left > STRIDE:
                    n = len(fillers) + len(pt)
                    quota = -(-n * STRIDE // max(tiles_left, 1))
                    pre_drained = min((quota + 1) // 2, n)
                    for _ in range(pre_drained):
                        drain_one()
                # scores for the next pair together: one 64x128-mode group
                # per step instead of two (array retiling drains the PE)
                if CFG["spair"]:
                    u = t0 + LOOK
                    if u + 1 < len(seq):
                        p_tiles.update(scores_pair(g, seq[u], seq[u + 1]))
                else:
                    for u in range(t0 + LOOK,
                                   min(t0 + LOOK + STRIDE, len(seq))):
                        p_tiles[seq[u]] = scores_unit(g, *seq[u])
                drain = 0
                for t in range(t0, min(t0 + STRIDE, len(seq))):
                    j, i = seq[t]
                    ntk = 4 * (j + 1)
                    pyA, pyB = ensure_py(j)
                    pv_unit(g, j, i, ntk, p_tiles.pop((j, i)), pyA, pyB)
                    tiles_left -= 1
                    if i == ntk - 1:
                        y_sb = norm(g, j, pyA, pyB)
                        last = (g == G - 1) and (j == NJ - 1)
                        fillers.extend(outproj_units(g, j, y_sb, last=last))
                        del pys[j]
                    elif CFG["interleave"]:
                        # drain fillers (out-proj first, then next-batch
                        # producers) at a rate that finishes by batch end;
                        # the last PV of a block is followed by norm() so
                        # the recips hit the DVE queue first (they gate the
                        # next block's PSUM), extra drains early cover the
                        # gap.
                        n = len(fillers) + len(pt)
                        k = -(-n // max(tiles_left, 1)) if tiles_left else n
                        if i < 2:
                            k += 1
                        drain += k
                drain = max(drain - pre_drained, 0)
                n = len(fillers) + len(pt)
                for _ in range(min(drain, n)):
                    drain_one()
            if not CFG["interleave"]:
                while fillers:
                    fillers.popleft()()
            # next-batch producers must be fully emitted before the next
            # batch's attention reads them (engines execute in program order)
            while pt:
                if fillers:
                    fillers.popleft()()
                pt.popleft()()
        while fillers:
            fillers.popleft()()


def build(reps=1):
    nc = bacc.Bacc()
    xT = nc.dram_tensor("xT", [C, BT], BF16, kind="ExternalInput")
    wqkvT = nc.dram_tensor("wqkvT", [C, F], BF16, kind="ExternalInput")
    bqkv = nc.dram_tensor("bqkv", [F], FP32, kind="ExternalInput")
    woutT = nc.dram_tensor("woutT", [CPC, C], BF16, kind="ExternalInput")
    outT = nc.dram_tensor("outT", [C, BT], BF16, kind="ExternalOutput")
    with tile.TileContext(nc) as tc:
        _emit(tc, nc, xT.ap(), wqkvT.ap(), bqkv.ap(), woutT.ap(), outT.ap(),
              reps=reps)
    nc.compile()
    return nc


def make_in_maps(x, attention_mask, Wqkv, bqkv, Wout):
    # attention_mask is all-ones per the problem spec (fill: ones) -- the
    # kernel bakes in causal-only masking.
    xT = np.ascontiguousarray(
        x.reshape(BT, C).T).astype(NPBF16)
    in_maps = []
    for c in range(NCORES):
        rows = np.r_[c * CPC:(c + 1) * CPC,
                     C + c * CPC:C + (c + 1) * CPC,
                     2 * C + c * CPC:2 * C + (c + 1) * CPC]
        wqkvT_c = np.ascontiguousarray(Wqkv[rows, :].T).astype(NPBF16)
        b_c = np.ascontiguousarray(bqkv[rows].astype(np.float32, copy=False))
        woutT_c = np.ascontiguousarray(
            Wout[:, c * CPC:(c + 1) * CPC].T).astype(NPBF16)
        in_maps.append({"xT": xT, "wqkvT": wqkvT_c, "bqkv": b_c,
                        "woutT": woutT_c})
    return in_maps


def kernel(x, attention_mask, Wqkv, bqkv, Wout, _trace=False):
    x = np.asarray(x)
    attention_mask = np.asarray(attention_mask)
    Wqkv = np.asarray(Wqkv)
    bqkv = np.asarray(bqkv)
    Wout = np.asarray(Wout)
    if "nc" not in _cached:
        _cached["nc"] = build()
    nc = _cached["nc"]
    in_maps = make_in_maps(x, attention_mask, Wqkv, bqkv, Wout)
    res = bass_utils.run_bass_kernel_spmd(
        nc, in_maps, core_ids=list(range(NCORES)), trace=_trace)
    acc = res.results[0]["outT"].astype(np.float32)
    for r in res.results[1:]:
        acc += r["outT"].astype(np.float32)
    out = np.ascontiguousarray(acc.T).reshape(B, T, C).astype(np.float32)
    if _trace:
        _cached["last_result"] = res
    return out

